# revision 1
# baseline (speedup 1.0000x reference)
# Causal self-attention (B=2, S=2048, D=1024, H=16) on 8 TRN2 NeuronCores.
#
# Sharding: core = (batch b, head-group hg) with 4 heads per core — data
# parallel on B (cores 0-3 = batch 0, cores 4-7 = batch 1), tensor parallel
# on heads within each batch group. Per core:
#   1. qkv^T projection for its 4 heads:  qkvT[768, 2048] = wqkv_s^T @ x_b^T
#   2. causal attention in scores^T layout (keys on partitions):
#        scoresT[k, q] = K^T.T @ Q^T ; exp on ACT (scale=1/8 fused, k-tiles
#        exp'd in pairs to amortize the ~352-cycle ACT op overhead);
#        diagonal-block masking via an upper-triangular mask multiply;
#        ctxT[d, q] accumulates with a ones-column appended to V so row 64
#        of the ctx psum is the softmax denominator.
#   3. normalize: broadcast sums via a K=1 outer-product matmul, then a
#      single DVE divide into bf16 ctx_sb
#   4. AllGather ctx^T over the 4-core batch group, split into two token
#      halves so gather #1 overlaps attention of the second half
#   5. out-projection for this core's 256 output columns (+bias), token
#      chunks ordered so the first half starts as soon as gather #1 lands
# Host side shards/pre-transposes inputs and concatenates the 8 output
# column-slices; no host arithmetic beyond dtype casts and transposes.

import numpy as np
import ml_dtypes

import concourse.bass as bass
import concourse.mybir as mybir
import concourse.tile as tile
from concourse import bacc
from concourse.bass_utils import run_bass_kernel_spmd
from concourse.masks import make_identity, make_upper_triangular

F32 = mybir.dt.float32
BF16 = mybir.dt.bfloat16

B, S, D, H, HD = 2, 2048, 1024, 16, 64
HG = 4                 # heads per core
DG = HG * HD           # 256 qkv cols per head-group
NCORES = 8
KT = 128               # key tile (partition dim of scoresT)
QC = 512               # query chunk (free dim of scoresT / psum width)
NKT = S // KT          # 16 key tiles
NQC = S // QC          # 4 query chunks
SM_SCALE = 1.0 / 8.0   # 1/sqrt(HD)

# dtype knobs (matmul operand / storage dtypes; psums always fp32)
XW_DT = BF16           # x, w_qkv, and the Q^T/K^T tiles (scores matmul)
V_DT = BF16            # V natural tiles (ctx matmul lhsT)
ATTN_DT = BF16         # exp(scores) tiles (ctx matmul rhs)
CC_DT = BF16           # allgathered ctx^T
WOUT_DT = BF16         # out-projection weights

_NP = {BF16: ml_dtypes.bfloat16, F32: np.float32}

LAST_RESULTS = None    # BassKernelResults of the most recent kernel() call
_NC_CACHE = {}
DEBUG_OUTPUTS = False  # add per-stage debug outputs (dbg_qk/dbg_v/dbg_ctx/...)


def _build_nc():
    nc = bacc.Bacc(
        trn_type="TRN2",
        target_bir_lowering=False,
        debug=False,
        num_devices=NCORES,
    )

    xT = nc.declare_dram_parameter("xT", [D, S], XW_DT, isOutput=False)
    wqkv = nc.declare_dram_parameter("wqkv", [D, 3 * DG], XW_DT, isOutput=False)
    bqkv = nc.declare_dram_parameter("bqkv", [128, 6], F32, isOutput=False)
    wout = nc.declare_dram_parameter("wout", [D, DG], WOUT_DT, isOutput=False)
    bout = nc.declare_dram_parameter("bout", [128, 2], F32, isOutput=False)
    outT = nc.declare_dram_parameter("outT", [DG, S], F32, isOutput=True)
    if DEBUG_OUTPUTS:
        dbg_qk = nc.declare_dram_parameter(
            "dbg_qk", [128, 4 * S], BF16, isOutput=True)
        dbg_v = nc.declare_dram_parameter(
            "dbg_v", [128, HG * NKT * (HD + 1)], BF16, isOutput=True)
        dbg_ctx = nc.declare_dram_parameter(
            "dbg_ctx", [128, 2 * S], BF16, isOutput=True)
        dbg_g = nc.declare_dram_parameter(
            "dbg_g", [128, 8 * S], BF16, isOutput=True)

    KC = D // 128  # 8 contraction chunks for the projections

    with tile.TileContext(nc) as tc:
        with tc.tile_pool(name="persist", bufs=1) as ps:
            # ---- constants ----
            identity = ps.tile([128, 128], XW_DT, tag="identity")
            make_identity(nc, identity)
            tri = ps.tile([128, 128], F32, tag="tri")
            make_upper_triangular(nc, tri, val=1.0, diag=True)
            tri_mm = ps.tile([128, 128], ATTN_DT, tag="tri_mm")
            nc.vector.tensor_copy(tri_mm, tri)
            ones1 = ps.tile([1, 64], ATTN_DT, tag="ones1")
            nc.vector.memset(ones1, 1.0)

            # ---- persistent SBUF tensors ----
            xT_sb = ps.tile([128, KC, S], XW_DT, tag="xT_sb")
            wqkv_sb = ps.tile([128, KC, 3 * DG], XW_DT, tag="wqkv_sb")
            bqkv_sb = ps.tile([128, 6], F32, tag="bqkv_sb")
            qk_sb = ps.tile([128, 4, S], XW_DT, tag="qk_sb")      # Q^T,K^T
            vT_sb = ps.tile([128, 2, S], V_DT, tag="vT_sb")       # V^T
            vnat_sb = ps.tile([128, HG, NKT, HD + 1], V_DT, tag="vnat_sb")
            ctx_sb = ps.tile([128, 2, S], CC_DT, tag="ctx_sb")    # normalized
            ctxg_sb = ps.tile([128, D // 128, S], CC_DT, tag="ctxg_sb")
            wout_sb = ps.tile([128, KC, DG], WOUT_DT, tag="wout_sb")
            bout_sb = ps.tile([128, 2], F32, tag="bout_sb")
            outT_sb = ps.tile([128, 2, S], F32, tag="outT_sb")

            # ---- load inputs ----
            xT_r = xT.rearrange("(c p) s -> c p s", p=128)
            wqkv_r = wqkv.rearrange("(c p) m -> c p m", p=128)
            wout_r = wout.rearrange("(c p) m -> c p m", p=128)
            for c in range(KC):
                nc.sync.dma_start(out=xT_sb[:, c, :], in_=xT_r[c])
                nc.sync.dma_start(out=wqkv_sb[:, c, :], in_=wqkv_r[c])
                nc.sync.dma_start(out=wout_sb[:, c, :], in_=wout_r[c])
            nc.sync.dma_start(out=bqkv_sb, in_=bqkv[:])
            nc.sync.dma_start(out=bout_sb, in_=bout[:])

            # ---- stages 1-4 interleaved per token chunk:
            # proj(n) -> V-transpose(n) -> attention(n) -> gather(n).
            # Projection of chunk n+1 overlaps attention of chunk n (which is
            # ACT-bound), and each quarter AllGather overlaps the next chunk.
            nc.vector.memset(vnat_sb, 1.0)
            with tc.tile_pool(name="dram", bufs=1, space="DRAM") as dram:

                cc_in = [dram.tile([DG, QC], CC_DT, tag=f"cc_in{q}",
                                   name=f"cc_in{q}") for q in range(NQC)]
                cc_out = [dram.tile([D, QC], CC_DT, tag=f"cc_out{q}",
                                    name=f"cc_out{q}") for q in range(NQC)]

                def proj_chunk(n):
                    # m-chunk order: q01 q23 k01 k23 v01 v23
                    for m in range(6):
                        pt = pps.tile([128, QC], F32, tag="proj")
                        for c in range(KC):
                            nc.tensor.matmul(
                                pt,
                                lhsT=wqkv_sb[:, c, m * 128:(m + 1) * 128],
                                rhs=xT_sb[:, c, n * QC:(n + 1) * QC],
                                start=(c == 0),
                                stop=(c == KC - 1),
                            )
                        if m < 4:
                            dst = qk_sb[:, m, n * QC:(n + 1) * QC]
                        else:
                            dst = vT_sb[:, m - 4, n * QC:(n + 1) * QC]
                        nc.vector.tensor_scalar_add(
                            dst, pt, bqkv_sb[:, m:m + 1])
                    # V natural (+ ones column) via PE transpose
                    for h in range(HG):
                        po = 64 * (h % 2)
                        mv = h // 2
                        for t in range(4 * n, 4 * n + 4):
                            tp = axp.tile([128, HD], V_DT, tag="tp", bufs=1)
                            nc.tensor.transpose(
                                tp,
                                vT_sb[po:po + 64, mv, t * KT:(t + 1) * KT],
                                identity[po:po + 64, po:po + 64],
                            )
                            nc.vector.tensor_copy(vnat_sb[:, h, t, 0:HD], tp)

                def attention_chunk(j):
                    sums_t = []
                    raws_t = []
                    for h in range(HG):
                        po = 64 * (h % 2)
                        mh = h // 2
                        cx = cxp.tile([HD + 1, QC], F32, tag="ctx")
                        n_kt = 4 * j + 4      # key tiles 0 .. 4j+3
                        for i in range(n_kt):
                            tshift = KT * i - QC * j
                            t0 = max(tshift, 0)
                            sc = scp.tile([128, QC], F32, tag="sc")
                            at = asb.tile([128, QC], ATTN_DT, tag="attn")
                            nc.tensor.matmul(
                                sc[:, t0:QC],
                                lhsT=qk_sb[po:po + 64, 2 + mh,
                                           i * KT:(i + 1) * KT],
                                rhs=qk_sb[po:po + 64, mh,
                                          j * QC + t0:(j + 1) * QC],
                                start=True, stop=True,
                            )
                            nc.scalar.activation(
                                at[:, t0:QC], sc[:, t0:QC],
                                mybir.ActivationFunctionType.Exp,
                                scale=SM_SCALE,
                            )
                            if tshift >= 0:   # diagonal: mask k > q
                                nc.vector.tensor_mul(
                                    at[:, t0:t0 + 128],
                                    at[:, t0:t0 + 128], tri_mm)
                            nc.tensor.matmul(
                                cx[:, t0:QC],
                                lhsT=vnat_sb[:, h, i, :],
                                rhs=at[:, t0:QC],
                                start=(i == 0),
                                stop=(i == n_kt - 1),
                            )
                        # evict raw ctx + sums fast (frees the psum so the
                        # next head's k-loop isn't gated on normalization)
                        sums = ssb.tile([1, QC], F32, tag="sums", bufs=4)
                        nc.vector.tensor_copy(sums, cx[HD:HD + 1, :])
                        raw = ssb.tile([64, QC], F32, tag="raw", bufs=4)
                        nc.vector.tensor_copy(raw, cx[0:HD, :])
                        sums_t.append(sums)
                        raws_t.append(raw)
                    # deferred normalization tail for the whole chunk
                    for h in range(HG):
                        po = 64 * (h % 2)
                        mh = h // 2
                        recip = ssb.tile([1, QC], ATTN_DT, tag="recip", bufs=4)
                        with nc.allow_low_precision(
                                reason="softmax denominator broadcast"):
                            nc.vector.reciprocal(recip, sums_t[h])
                        bc = axp.tile([64, QC], F32, tag="bc", bufs=1)
                        nc.tensor.matmul(
                            bc, lhsT=ones1, rhs=recip, start=True, stop=True)
                        nc.vector.tensor_mul(
                            ctx_sb[po:po + 64, mh, j * QC:(j + 1) * QC],
                            raws_t[h], bc)

                def gather_chunk(q):
                    lo = q * QC
                    cc_in_r = cc_in[q].rearrange("(c p) s -> c p s", p=128)
                    for c in range(2):
                        nc.sync.dma_start(
                            out=cc_in_r[c], in_=ctx_sb[:, c, lo:lo + QC])
                    nc.gpsimd.collective_compute(
                        "AllGather",
                        mybir.AluOpType.bypass,
                        replica_groups=[[0, 1, 2, 3], [4, 5, 6, 7]],
                        ins=[cc_in[q][:].opt()],
                        outs=[cc_out[q][:].opt()],
                    )
                    cc_out_r = cc_out[q].rearrange("(c p) s -> c p s", p=128)
                    for c in range(D // 128):
                        nc.sync.dma_start(
                            out=ctxg_sb[:, c, lo:lo + QC], in_=cc_out_r[c])

                def out_proj_chunk(n, opp):
                    for mo in range(2):
                        pt = opp.tile([128, QC], F32, tag="op")
                        for c in range(KC):
                            nc.tensor.matmul(
                                pt,
                                lhsT=wout_sb[:, c, mo * 128:(mo + 1) * 128],
                                rhs=ctxg_sb[:, c, n * QC:(n + 1) * QC],
                                start=(c == 0),
                                stop=(c == KC - 1),
                            )
                        nc.vector.tensor_scalar_add(
                            outT_sb[:, mo, n * QC:(n + 1) * QC], pt,
                            bout_sb[:, mo:mo + 1])

                with tc.tile_pool(name="proj_ps", bufs=1, space="PSUM") as pps, \
                     tc.tile_pool(name="aux_ps", bufs=1, space="PSUM") as axp, \
                     tc.tile_pool(name="sc_ps", bufs=2, space="PSUM") as scp, \
                     tc.tile_pool(name="ctx_ps", bufs=2, space="PSUM") as cxp, \
                     tc.tile_pool(name="op_ps", bufs=1, space="PSUM") as opp, \
                     tc.tile_pool(name="attn_sb", bufs=3) as asb, \
                     tc.tile_pool(name="small_sb", bufs=2) as ssb:
                    for n in range(NQC):
                        proj_chunk(n)
                        attention_chunk(n)
                        gather_chunk(n)
                        # out-proj of chunk n fills PE gaps of chunk n+1's
                        # attention once gather n lands
                        out_proj_chunk(n, opp)
                    outT_r = outT.rearrange("(c p) s -> c p s", p=128)
                    for c in range(2):
                        nc.sync.dma_start(out=outT_r[c], in_=outT_sb[:, c, :])

            if DEBUG_OUTPUTS:
                nc.sync.dma_start(
                    out=dbg_qk[:], in_=qk_sb.rearrange("p c s -> p (c s)"))
                nc.sync.dma_start(
                    out=dbg_v[:], in_=vnat_sb.rearrange("p h t d -> p (h t d)"))
                nc.sync.dma_start(
                    out=dbg_ctx[:], in_=ctx_sb.rearrange("p c s -> p (c s)"))
                nc.sync.dma_start(
                    out=dbg_g[:], in_=ctxg_sb.rearrange("p c s -> p (c s)"))

    nc.compile()
    return nc


def get_nc():
    if "nc" not in _NC_CACHE:
        _NC_CACHE["nc"] = _build_nc()
    return _NC_CACHE["nc"]


def make_in_maps(x, w_qkv, b_qkv, w_out, b_out):
    x = np.asarray(x, np.float32)
    w_qkv = np.asarray(w_qkv, np.float32)
    b_qkv = np.asarray(b_qkv, np.float32)
    w_out = np.asarray(w_out, np.float32)
    b_out = np.asarray(b_out, np.float32)

    xw_np = _NP[XW_DT]
    wout_np = _NP[WOUT_DT]

    xT = [np.ascontiguousarray(x[b].T).astype(xw_np) for b in range(B)]
    in_maps = []
    for core in range(NCORES):
        b, hg = core // HG, core % HG
        sl = slice(hg * DG, (hg + 1) * DG)
        wq = w_qkv[:, sl]
        wk = w_qkv[:, D + hg * DG:D + (hg + 1) * DG]
        wv = w_qkv[:, 2 * D + hg * DG:2 * D + (hg + 1) * DG]
        wqkv_s = np.ascontiguousarray(
            np.concatenate([wq, wk, wv], axis=1)).astype(xw_np)
        bq = np.concatenate(
            [b_qkv[sl], b_qkv[D + hg * DG:D + (hg + 1) * DG],
             b_qkv[2 * D + hg * DG:2 * D + (hg + 1) * DG]])
        in_maps.append({
            "xT": xT[b],
            "wqkv": wqkv_s,
            "bqkv": np.ascontiguousarray(bq.reshape(6, 128).T).astype(np.float32),
            "wout": np.ascontiguousarray(w_out[:, sl]).astype(wout_np),
            "bout": np.ascontiguousarray(
                b_out[sl].reshape(2, 128).T).astype(np.float32),
        })
    return in_maps


def assemble_output(results):
    out = np.empty((B, S, D), np.float32)
    for core in range(NCORES):
        b, hg = core // HG, core % HG
        out[b, :, hg * DG:(hg + 1) * DG] = results[core]["outT"].T
    return out


def kernel(x, w_qkv, b_qkv, w_out, b_out):
    global LAST_RESULTS
    in_maps = make_in_maps(x, w_qkv, b_qkv, w_out, b_out)
    nc = get_nc()
    res = run_bass_kernel_spmd(nc, in_maps, list(range(NCORES)))
    LAST_RESULTS = res
    return assemble_output(res.results)



# revision 27
# speedup vs baseline: 1.3923x; 1.3923x over previous
# Causal self-attention (B=2, S=2048, D=1024, H=16) on 8 TRN2 NeuronCores.
#
# Sharding: core = (batch b, head-group hg) with 4 heads per core — data
# parallel on B (cores 0-3 = batch 0, cores 4-7 = batch 1), tensor parallel
# on heads within each batch group. Per core:
#   1. Q/K projection (qk^T layout, tokens on free dim) for its 4 heads;
#      V projected directly in natural [token, dim] layout (lhsT = xT tile,
#      rhs = w_v) — no PE transposes; V bias via a K=1 ones-row matmul.
#   2. causal attention with the head PAIR interleaved in the k-loop:
#      the two heads of a pair live at partitions 0-63 / 64-127, so their
#      K=64 scores matmuls issue back-to-back and run concurrently in
#      different PE row groups; one Exp instruction covers both heads'
#      score banks ([128, 2, 512-t0] AP) halving ACT instruction count.
#      ctx^T accumulates with a ones-column appended to V so row 64 of the
#      ctx psum is the softmax denominator.
#   3. normalize: reciprocal_approx_fast on the denominator row (NOT the
#      56x slower iterative DVE reciprocal), broadcast via a K=1 matmul,
#      one DVE multiply into bf16 ctx_sb.
#   4. AllGather ctx^T over the 4-core batch group per 512-token chunk;
#      out-projection of chunk n is emitted AFTER attention of chunk n+1
#      so the gather wait never blocks queued PE work (engine FIFOs are
#      strict in-order). A dummy warmup AllGather absorbs the collective
#      cold-start during the input load phase.
# Host side shards/pre-transposes inputs and concatenates the 8 output
# column-slices; no host arithmetic beyond dtype casts and transposes.

import numpy as np
import ml_dtypes

import concourse.bass as bass
import concourse.mybir as mybir
import concourse.tile as tile
from concourse import bacc
from concourse.bass_utils import run_bass_kernel_spmd
from concourse.masks import make_upper_triangular

F32 = mybir.dt.float32
BF16 = mybir.dt.bfloat16

B, S, D, H, HD = 2, 2048, 1024, 16, 64
HG = 4                 # heads per core
DG = HG * HD           # 256 qkv cols per head-group
NCORES = 8
KT = 128               # key tile (partition dim of scoresT)
QC = 512               # query chunk (free dim of scoresT / psum width)
NKT = S // KT          # 16 key tiles
NQC = S // QC          # 4 query chunks
VW = HD + 2            # vnat stride per head (64 dims + ones col + pad,
                       # 66*2B keeps strided copy segments 4B-aligned)
SM_SCALE = 1.0 / 8.0   # 1/sqrt(HD)

# dtype knobs (matmul operand / storage dtypes; psums always fp32)
XW_DT = BF16           # x, w_qkv, and the Q^T/K^T tiles (scores matmul)
V_DT = BF16            # V natural tiles (ctx matmul lhsT)
ATTN_DT = BF16         # exp(scores) tiles (ctx matmul rhs)
CC_DT = BF16           # allgathered ctx^T
WOUT_DT = BF16         # out-projection weights

_NP = {BF16: ml_dtypes.bfloat16, F32: np.float32}

LAST_RESULTS = None    # BassKernelResults of the most recent kernel() call
_NC_CACHE = {}
DEBUG_OUTPUTS = False  # add per-stage debug outputs (dbg_qk/dbg_v/dbg_ctx/...)

KC = D // 128          # 8 contraction chunks for the projections


def _patch_act_tables():
    """Force Exp AND Ln onto the one set that has both
    (natural_log_exp_and_others) so the table-load pass emits a single
    load instead of thrashing between exp_and_others and natural_log
    (~2.7us per switch, 16 switches). Positions are preserved — the
    emitted act_func_set_id indexes the full act_info list."""
    import concourse.bacc as _bacc
    if getattr(_bacc, "_act_tables_patched", False):
        return
    orig = _bacc.get_activation_tables

    def patched(module_arch):
        tables = orig(module_arch)
        exp = mybir.ActivationFunctionType.Exp
        ln = mybir.ActivationFunctionType.Ln
        return {
            name: (set() if (name != "natural_log_exp_and_others"
                             and (exp in fns or ln in fns)) else fns)
            for name, fns in tables.items()
        }

    _bacc.get_activation_tables = patched
    _bacc._act_tables_patched = True


def _build_nc():
    _patch_act_tables()
    nc = bacc.Bacc(
        trn_type="TRN2",
        target_bir_lowering=False,
        debug=False,
        num_devices=NCORES,
    )

    xT = nc.declare_dram_parameter("xT", [D, S], XW_DT, isOutput=False)
    wqk = nc.declare_dram_parameter("wqk", [D, 2 * DG], XW_DT, isOutput=False)
    wv = nc.declare_dram_parameter("wv", [D, DG], XW_DT, isOutput=False)
    bqk = nc.declare_dram_parameter("bqk", [128, 4], F32, isOutput=False)
    bvrow = nc.declare_dram_parameter("bvrow", [1, DG], XW_DT, isOutput=False)
    wout = nc.declare_dram_parameter("wout", [D, DG], WOUT_DT, isOutput=False)
    bout = nc.declare_dram_parameter("bout", [128, 2], F32, isOutput=False)
    outT = nc.declare_dram_parameter("outT", [DG, S], F32, isOutput=True)
    if DEBUG_OUTPUTS:
        dbg_qk = nc.declare_dram_parameter(
            "dbg_qk", [128, 4 * S], BF16, isOutput=True)
        dbg_v = nc.declare_dram_parameter(
            "dbg_v", [128, NKT * HG * VW], BF16, isOutput=True)
        dbg_ctx = nc.declare_dram_parameter(
            "dbg_ctx", [128, 2 * S], BF16, isOutput=True)
        dbg_g = nc.declare_dram_parameter(
            "dbg_g", [128, 8 * S], BF16, isOutput=True)
        dbg_at = nc.declare_dram_parameter(
            "dbg_at", [128, 2 * QC], BF16, isOutput=True)
        dbg_rs = nc.declare_dram_parameter(
            "dbg_rs", [128, 16 * QC], F32, isOutput=True)

    with tile.TileContext(nc) as tc:
        with tc.tile_pool(name="persist", bufs=1) as ps:
            # ---- constants ----
            tri = ps.tile([128, 128], F32, tag="tri")
            make_upper_triangular(nc, tri, val=1.0, diag=True)
            tri_mm = ps.tile([128, 128], ATTN_DT, tag="tri_mm")
            nc.vector.tensor_copy(tri_mm, tri)
            ones1 = ps.tile([1, 128], ATTN_DT, tag="ones1")
            nc.vector.memset(ones1, 1.0)
            onesp = ps.tile([128, 64], ATTN_DT, tag="onesp")
            nc.vector.memset(onesp, 1.0)

            # ---- persistent SBUF tensors ----
            xT_sb = ps.tile([128, KC, S], XW_DT, tag="xT_sb")
            wqk_sb = ps.tile([128, KC, 2 * DG], XW_DT, tag="wqk_sb")
            wv_sb = ps.tile([128, KC, DG], XW_DT, tag="wv_sb")
            bqk_sb = ps.tile([128, 4], F32, tag="bqk_sb")
            bvrow_sb = ps.tile([1, DG], XW_DT, tag="bvrow_sb")
            qk_sb = ps.tile([128, 4, S], XW_DT, tag="qk_sb")      # Q^T,K^T
            vnat_sb = ps.tile([128, NKT, HG, VW], V_DT, tag="vnat_sb")
            ctx_sb = ps.tile([128, 2, S], CC_DT, tag="ctx_sb")    # normalized
            ctxg_sb = ps.tile([128, D // 128, S], CC_DT, tag="ctxg_sb")
            wout_sb = ps.tile([128, KC, DG], WOUT_DT, tag="wout_sb")
            bout_sb = ps.tile([128, 2], F32, tag="bout_sb")
            outT_sb = ps.tile([128, 2, S], F32, tag="outT_sb")

            nc.vector.memset(vnat_sb, 1.0)   # bakes the ones columns

            if DEBUG_OUTPUTS:
                dbg_at_sb = ps.tile([128, 2, QC], ATTN_DT, tag="dbg_at_sb")
                dbg_rs_sb = ps.tile([128, 16, QC], F32, tag="dbg_rs_sb")
                nc.vector.memset(dbg_at_sb, 0.0)
                nc.vector.memset(dbg_rs_sb, 0.0)

            with tc.tile_pool(name="dram", bufs=1, space="DRAM") as dram:
                warm_in = dram.tile([128, 16], CC_DT, tag="warm_in",
                                    name="warm_in")
                warm_out = dram.tile([512, 16], CC_DT, tag="warm_out",
                                     name="warm_out")
                cc_in = [dram.tile([DG, QC], CC_DT, tag=f"cc_in{q}",
                                   name=f"cc_in{q}") for q in range(NQC)]
                cc_out = [dram.tile([D, QC], CC_DT, tag=f"cc_out{q}",
                                    name=f"cc_out{q}") for q in range(NQC)]

                # warmup collective: absorbs the ncfw cold-start + entry
                # barrier while the input DMAs stream in
                warm_sb = ps.tile([128, 16], CC_DT, tag="warm_sb")
                nc.vector.memset(warm_sb, 0.0)
                nc.sync.dma_start(out=warm_in[:], in_=warm_sb)
                nc.gpsimd.collective_compute(
                    "AllGather",
                    mybir.AluOpType.bypass,
                    replica_groups=[[0, 1, 2, 3], [4, 5, 6, 7]],
                    ins=[warm_in[:].opt()],
                    outs=[warm_out[:].opt()],
                )

                # ---- load inputs (chunk-0-critical pieces first) ----
                xT_r = xT.rearrange("(c p) s -> c p s", p=128)
                wqk_r = wqk.rearrange("(c p) m -> c p m", p=128)
                wv_r = wv.rearrange("(c p) m -> c p m", p=128)
                wout_r = wout.rearrange("(c p) m -> c p m", p=128)
                for c in range(KC):
                    nc.sync.dma_start(out=xT_sb[:, c, 0:QC],
                                      in_=xT_r[c][:, 0:QC])
                nc.sync.dma_start(out=bqk_sb, in_=bqk[:])
                nc.sync.dma_start(out=bvrow_sb, in_=bvrow[:])
                for c in range(KC):
                    nc.sync.dma_start(out=wqk_sb[:, c, :], in_=wqk_r[c])
                    nc.sync.dma_start(out=wv_sb[:, c, :], in_=wv_r[c])
                for c in range(KC):
                    nc.sync.dma_start(out=xT_sb[:, c, QC:S],
                                      in_=xT_r[c][:, QC:S])
                    nc.sync.dma_start(out=wout_sb[:, c, :], in_=wout_r[c])
                nc.sync.dma_start(out=bout_sb, in_=bout[:])

                def proj_chunk(n, gmm):
                    # Q/K for this chunk's tokens: m-chunk order q01 q23
                    # k01 k23 (heads 0,1 on partitions 0-63 / 64-127 of
                    # pair tiles)
                    for m in range(4):
                        pt = gmm.tile([128, QC], F32, tag="gemm")
                        for c in range(KC):
                            nc.tensor.matmul(
                                pt,
                                lhsT=wqk_sb[:, c, m * 128:(m + 1) * 128],
                                rhs=xT_sb[:, c, n * QC:(n + 1) * QC],
                                start=(c == 0),
                                stop=(c == KC - 1),
                            )
                        nc.vector.tensor_scalar_add(
                            qk_sb[:, m, n * QC:(n + 1) * QC], pt,
                            bqk_sb[:, m:m + 1])
                    # V natural directly: out[token, vdim], bias via K=1 mm
                    for t in range(4 * n, 4 * n + 4):
                        vt = gmm.tile([128, DG], F32, tag="gemm")
                        nc.tensor.matmul(
                            vt, lhsT=ones1[:, 0:128], rhs=bvrow_sb[:],
                            start=True, stop=False)
                        for c in range(KC):
                            nc.tensor.matmul(
                                vt,
                                lhsT=xT_sb[:, c, t * KT:(t + 1) * KT],
                                rhs=wv_sb[:, c, :],
                                start=False,
                                stop=(c == KC - 1),
                            )
                        nc.vector.tensor_copy(
                            vnat_sb[:, t, :, 0:HD],
                            vt.rearrange("p (h d) -> p h d", h=HG))

                def attention_chunk(j, gmm, scp, cxp, asb, ssb):
                    n_kt = 4 * j + 4      # key tiles 0 .. 4j+3
                    raws = []
                    recips = []
                    for p in range(2):    # head pair p: heads 2p, 2p+1
                        sums2 = ssb.tile([33, QC], F32, tag="sums2",
                                         bufs=2)
                        cx = [cxp.tile([HD + 1, QC], F32, tag="cx",
                                       name=f"cx{z}")
                              for z in range(2)]
                        for i in range(n_kt):
                            tshift = KT * i - QC * j
                            t0 = max(tshift, 0)
                            sc = scp.tile([128, 2, QC], F32, tag="sc")
                            at = asb.tile([128, 2, QC], ATTN_DT, tag="attn")
                            for z in range(2):   # heads at po 0 / 64
                                po = 64 * z
                                nc.tensor.matmul(
                                    sc[:, z, t0:QC],
                                    lhsT=qk_sb[po:po + 64, 2 + p,
                                               i * KT:(i + 1) * KT],
                                    rhs=qk_sb[po:po + 64, p,
                                              j * QC + t0:(j + 1) * QC],
                                    start=True, stop=True,
                                )
                            nc.scalar.activation(
                                at[:, :, t0:QC], sc[:, :, t0:QC],
                                mybir.ActivationFunctionType.Exp,
                                scale=SM_SCALE,
                            )
                            if tshift >= 0:   # diagonal: mask k > q
                                for z in range(2):
                                    nc.vector.tensor_mul(
                                        at[:, z, t0:t0 + 128],
                                        at[:, z, t0:t0 + 128], tri_mm)
                            if DEBUG_OUTPUTS and j == 0 and p == 0 and i == 0:
                                nc.vector.tensor_copy(dbg_at_sb, at)
                            for z in range(2):
                                h = 2 * p + z
                                nc.tensor.matmul(
                                    cx[z][:, t0:QC],
                                    lhsT=vnat_sb[:, i, h, 0:HD + 1],
                                    rhs=at[:, z, t0:QC],
                                    start=(i == 0),
                                    stop=(i == n_kt - 1),
                                )
                        # evict raw ctx + sums in one copy per head (frees
                        # the psum banks for the next pair); sums rows are
                        # parked at partitions 32h of a shared tile for the
                        # chunk-batched reciprocal
                        for z in range(2):
                            h = 2 * p + z
                            rs = ssb.tile([HD + 1, QC], F32, tag="rs",
                                          bufs=4)
                            nc.vector.tensor_copy(rs, cx[z][:])
                            nc.vector.tensor_copy(
                                sums2[32 * z:32 * z + 1, :],
                                rs[HD:HD + 1, :])
                            if DEBUG_OUTPUTS:
                                hx = j * 4 + 2 * p + z
                                nc.vector.tensor_copy(
                                    dbg_rs_sb[0:HD + 1, hx, :], rs)
                            raws.append(rs)
                        # pair-batched reciprocal on ACT: 1/s = exp(-ln s);
                        # Ln and Exp share one activation table set
                        lnt = ssb.tile([33, QC], F32, tag="lnt", bufs=2)
                        nc.scalar.activation(
                            lnt, sums2[0:33, :],
                            mybir.ActivationFunctionType.Ln)
                        recip2 = ssb.tile([33, QC], ATTN_DT, tag="recip2",
                                          bufs=2)
                        with nc.allow_low_precision(
                                reason="softmax denominator broadcast"):
                            nc.scalar.activation(
                                recip2, lnt,
                                mybir.ActivationFunctionType.Exp,
                                scale=-1.0)
                        recips.append(recip2)
                    for h in range(4):
                        p, z = h // 2, h % 2
                        if DEBUG_OUTPUTS:
                            nc.vector.tensor_copy(
                                dbg_rs_sb[96:97, j * 4 + h, :],
                                recips[p][32 * z:32 * z + 1, :])
                        bc = gmm.tile([64, QC], F32, tag="gemm")
                        nc.tensor.matmul(
                            bc, lhsT=onesp[32 * z:32 * z + 1, :],
                            rhs=recips[p][32 * z:32 * z + 1, :],
                            start=True, stop=True)
                        nc.vector.tensor_mul(
                            ctx_sb[64 * z:64 * z + 64, p,
                                   j * QC:(j + 1) * QC],
                            raws[h][0:HD, :], bc)

                def gather_chunk(q):
                    lo = q * QC
                    cc_in_r = cc_in[q].rearrange("(c p) s -> c p s", p=128)
                    for c in range(2):
                        nc.sync.dma_start(
                            out=cc_in_r[c], in_=ctx_sb[:, c, lo:lo + QC])
                    nc.gpsimd.collective_compute(
                        "AllGather",
                        mybir.AluOpType.bypass,
                        replica_groups=[[0, 1, 2, 3], [4, 5, 6, 7]],
                        ins=[cc_in[q][:].opt()],
                        outs=[cc_out[q][:].opt()],
                    )
                    cc_out_r = cc_out[q].rearrange("(c p) s -> c p s", p=128)
                    for c in range(D // 128):
                        nc.sync.dma_start(
                            out=ctxg_sb[:, c, lo:lo + QC], in_=cc_out_r[c])

                outT_r = outT.rearrange("(c p) s -> c p s", p=128)

                def out_proj_chunk(n, gmm):
                    for mo in range(2):
                        pt = gmm.tile([128, QC], F32, tag="gemm")
                        for c in range(KC):
                            nc.tensor.matmul(
                                pt,
                                lhsT=wout_sb[:, c, mo * 128:(mo + 1) * 128],
                                rhs=ctxg_sb[:, c, n * QC:(n + 1) * QC],
                                start=(c == 0),
                                stop=(c == KC - 1),
                            )
                        nc.vector.tensor_scalar_add(
                            outT_sb[:, mo, n * QC:(n + 1) * QC], pt,
                            bout_sb[:, mo:mo + 1])
                    for c in range(2):
                        nc.sync.dma_start(
                            out=outT_r[c][:, n * QC:(n + 1) * QC],
                            in_=outT_sb[:, c, n * QC:(n + 1) * QC])

                with tc.tile_pool(name="gemm_ps", bufs=2, space="PSUM") as gmm, \
                     tc.tile_pool(name="sc_ps", bufs=2, space="PSUM") as scp, \
                     tc.tile_pool(name="ctx_ps", bufs=2, space="PSUM") as cxp, \
                     tc.tile_pool(name="attn_sb", bufs=3) as asb, \
                     tc.tile_pool(name="small_sb", bufs=2) as ssb:
                    for n in range(NQC):
                        proj_chunk(n, gmm)
                        attention_chunk(n, gmm, scp, cxp, asb, ssb)
                        gather_chunk(n)
                        # out-proj of chunk n-1 gap-fills chunk n's
                        # attention once gather n-1 lands (emitted after so
                        # the gather wait never heads the PE FIFO)
                        if n > 0:
                            out_proj_chunk(n - 1, gmm)
                    out_proj_chunk(NQC - 1, gmm)

            if DEBUG_OUTPUTS:
                nc.sync.dma_start(
                    out=dbg_qk[:], in_=qk_sb.rearrange("p c s -> p (c s)"))
                nc.sync.dma_start(
                    out=dbg_v[:],
                    in_=vnat_sb.rearrange("p t h d -> p (t h d)"))
                nc.sync.dma_start(
                    out=dbg_ctx[:], in_=ctx_sb.rearrange("p c s -> p (c s)"))
                nc.sync.dma_start(
                    out=dbg_g[:], in_=ctxg_sb.rearrange("p c s -> p (c s)"))
                nc.sync.dma_start(
                    out=dbg_at[:],
                    in_=dbg_at_sb.rearrange("p c s -> p (c s)"))
                nc.sync.dma_start(
                    out=dbg_rs[:],
                    in_=dbg_rs_sb.rearrange("p c s -> p (c s)"))

    nc.compile()
    return nc


def get_nc():
    if "nc" not in _NC_CACHE:
        _NC_CACHE["nc"] = _build_nc()
    return _NC_CACHE["nc"]


def make_in_maps(x, w_qkv, b_qkv, w_out, b_out):
    x = np.asarray(x, np.float32)
    w_qkv = np.asarray(w_qkv, np.float32)
    b_qkv = np.asarray(b_qkv, np.float32)
    w_out = np.asarray(w_out, np.float32)
    b_out = np.asarray(b_out, np.float32)

    xw_np = _NP[XW_DT]
    wout_np = _NP[WOUT_DT]

    xT = [np.ascontiguousarray(x[b].T).astype(xw_np) for b in range(B)]
    in_maps = []
    for core in range(NCORES):
        b, hg = core // HG, core % HG
        sl = slice(hg * DG, (hg + 1) * DG)
        wq = w_qkv[:, sl]
        wk = w_qkv[:, D + hg * DG:D + (hg + 1) * DG]
        wv = w_qkv[:, 2 * D + hg * DG:2 * D + (hg + 1) * DG]
        bqk = np.concatenate(
            [b_qkv[sl], b_qkv[D + hg * DG:D + (hg + 1) * DG]])
        bv = b_qkv[2 * D + hg * DG:2 * D + (hg + 1) * DG]
        in_maps.append({
            "xT": xT[b],
            "wqk": np.ascontiguousarray(
                np.concatenate([wq, wk], axis=1)).astype(xw_np),
            "wv": np.ascontiguousarray(wv).astype(xw_np),
            "bqk": np.ascontiguousarray(
                bqk.reshape(4, 128).T).astype(np.float32),
            "bvrow": np.ascontiguousarray(bv.reshape(1, DG)).astype(xw_np),
            "wout": np.ascontiguousarray(w_out[:, sl]).astype(wout_np),
            "bout": np.ascontiguousarray(
                b_out[sl].reshape(2, 128).T).astype(np.float32),
        })
    return in_maps


def assemble_output(results):
    out = np.empty((B, S, D), np.float32)
    for core in range(NCORES):
        b, hg = core // HG, core % HG
        out[b, :, hg * DG:(hg + 1) * DG] = results[core]["outT"].T
    return out


def kernel(x, w_qkv, b_qkv, w_out, b_out):
    global LAST_RESULTS
    in_maps = make_in_maps(x, w_qkv, b_qkv, w_out, b_out)
    nc = get_nc()
    res = run_bass_kernel_spmd(nc, in_maps, list(range(NCORES)))
    LAST_RESULTS = res
    return assemble_output(res.results)
